# revision 11
# baseline (speedup 1.0000x reference)
"""Trainium2 Bass kernel for a GCN message-passing layer.

Reference computation (per node i):
    out[i] = sum_j edges[i,j] * (w1 @ concat(x[j], dist[i,j])) + w2 @ x[i]
which factors into:
    xmsg = x @ w1x.T                       (w1x = w1[:, :128])
    agg  = edges @ xmsg                    (big GEMM, contraction over j)
    dw   = einsum('ij,ijc->ic', edges, dist)
    out  = agg + dw @ w1d.T + x @ w2.T     (w1d = w1[:, 128:130])

Sharding: rows i (targets) split across 8 NeuronCores; x/w1/w2 replicated.

Device strategy (tolerance is 2e-2 rel-L2; this kernel lands ~5e-3):
  - edges stream as bf16, dist channels as fp8-e4m3 (dist only feeds the
    small dw term).  Per-core HBM traffic: ~100 MB fp32 -> ~34 MB.
  - edges/dist are PRE-TRANSPOSED on the host to [j, i] layout and
    PAIR-PACKED: two 128-row j-tiles share one DMA so every per-partition
    run is 4 KB (2 KB runs measured only ~250 GB/s) and the HWDGE
    trigger count halves.
  - out^T[f, i] = sum_j xmsg[j, f] * E^T[j, i] accumulates in a single
    [128, 1024] fp32 PSUM tile via 512-col bf16 matmuls (1 cycle/col;
    one matmul may not span a PSUM bank).
  - dist term: DVE computes prod = E^T (*) D^T for both channels of both
    tiles of a pair in ONE tensor_tensor (broadcast AP on E^T, bf16 2x
    perf mode), and the PE folds the j-reduction AND the w1d GEMM into
    the same PSUM accumulation via broadcast-w1d stationaries
    (w1db_c[j, f] = w1d[f, c] for all j), batched across pairs so one
    LDWEIGHTS covers 8 dist matmuls.
  - fp8->bf16 upconversion of dist: ~1/3 of pairs (incl. the last two,
    for a short tail) ride a GPSIMD SWDGE dma that casts in-flight (fp8
    read from HBM, bf16 written to SBUF -- zero engine cost); the rest
    run on the otherwise-idle ScalarE activation pipe.
  - the xmsg prologue is INTERLEAVED into the main loop (one 4-chunk
    PSUM batch per pair-group, 2 groups of lookahead) so neither the
    ScalarE nor the DVE queue is head-of-line blocked at t=0 and the
    stream starts consuming immediately.
"""

import os

import numpy as np
import ml_dtypes

import concourse.bacc as bacc
import concourse.mybir as mybir
from concourse.tile import TileContext

F32 = mybir.dt.float32
BF16 = mybir.dt.bfloat16
FP8 = mybir.dt.float8e4
P = 128

# problem dims (hardcoded per contract)
N_FULL = 8192
F_IN = 128
F_OUT = 128
N_CORES = 8

# tunables
CAST_EVERY = 3      # every k-th pair loads dist via casting SWDGE dma
DIST_BATCH = 2      # pairs sharing one w1db stationary load
STREAM_BUFS = 6     # pair-tiles buffered per stream tag

LAST_RESULT = None  # BassKernelResults of the most recent kernel() call


def build(n=N_FULL, rows=N_FULL // N_CORES):
    """Build the per-core SPMD Bass program.

    n:    number of source nodes j (contraction dim, partition-tiled)
    rows: number of target rows i this core handles (free dim of out^T)
    """
    f = F_IN
    assert n % (2 * P) == 0 and rows == 1024
    n_pair = n // (2 * P)  # pair-packed j-tiles streamed from HBM
    n_grp = n_pair // DIST_BATCH
    XCH = n // 4           # x^T prologue piece width (j columns)

    nc = bacc.Bacc()
    eP_d = nc.declare_dram_parameter("eP", [n_pair, P, 2, rows], BF16,
                                     isOutput=False)
    dP_d = nc.declare_dram_parameter("dP", [n_pair, P, 2, 2, rows], FP8,
                                     isOutput=False)
    xT_d = nc.declare_dram_parameter("xT", [f, n], BF16, isOutput=False)
    xTs_d = nc.declare_dram_parameter("xT_self", [f, rows], BF16, isOutput=False)
    w1xT_d = nc.declare_dram_parameter("w1xT", [f, F_OUT], BF16, isOutput=False)
    w2T_d = nc.declare_dram_parameter("w2T", [f, F_OUT], BF16, isOutput=False)
    w1db_d = nc.declare_dram_parameter("w1db", [P, 2, F_OUT], BF16, isOutput=False)
    o_d = nc.declare_dram_parameter("outT", [F_OUT, rows], F32, isOutput=True)

    def is_cast(q):
        return q % CAST_EVERY == CAST_EVERY - 1 or q >= n_pair - 2

    with TileContext(nc) as tc:
        with (
            tc.tile_pool(name="const", bufs=1) as cpool,
            tc.tile_pool(name="stream", bufs=2) as pool,
            tc.tile_pool(name="psum", bufs=2, space="PSUM") as pp,
        ):
            def load_pair(q):
                """Returns (et2, db_or_d8). et2: [P, 2jt, rows]."""
                et2 = pool.tile([P, 2, rows], BF16, tag="E", bufs=STREAM_BUFS,
                                name=f"et{q}")
                nc.sync.dma_start(et2, eP_d[q])
                if is_cast(q):
                    # casting SWDGE load: fp8 in HBM -> bf16 in SBUF
                    db = pool.tile([P, 2, 2, rows], BF16, tag="DC", bufs=2,
                                   name=f"dbc{q}")
                    nc.gpsimd.dma_start(db, dP_d[q])
                    return et2, db
                d8 = pool.tile([P, 2, 2, rows], FP8, tag="D", bufs=STREAM_BUFS,
                               name=f"d8_{q}")
                nc.sync.dma_start(d8, dP_d[q])
                return et2, d8

            # issue the first pairs' loads before anything else so the
            # rings start streaming at t=0
            pre = {q: load_pair(q) for q in range(5)}

            # ---------------- prologue (scalar HWDGE ring) ----------------
            # tiny weights FIRST (w1xT gates the whole xmsg -> agg chain),
            # then the x^T pieces
            w1xT = cpool.tile([f, F_OUT], BF16)
            nc.scalar.dma_start(w1xT, w1xT_d[:, :])
            w2T = cpool.tile([f, F_OUT], BF16)
            nc.scalar.dma_start(w2T, w2T_d[:, :])
            w1db = cpool.tile([P, 2, F_OUT], BF16)
            nc.scalar.dma_start(w1db, w1db_d[:, :, :])
            xTs_sb = cpool.tile([f, rows], BF16)
            nc.scalar.dma_start(xTs_sb, xTs_d[:, :])
            xTp = []
            for b in range(n // XCH):
                t = cpool.tile([f, XCH], BF16, name=f"xTp{b}")
                nc.scalar.dma_start(t, xT_d[:, b * XCH : (b + 1) * XCH])
                xTp.append(t)

            xmsg = cpool.tile([P, n // P, f], BF16)

            def xmsg_stage(qq):
                """One 4-chunk xmsg batch: 4 matmuls + one copy."""
                xm = pp.tile([P, 4, f], F32, tag="xstage", bufs=4)
                for r in range(4):
                    ch = 4 * qq + r
                    b, off = divmod(ch * P, XCH)
                    nc.tensor.matmul(
                        xm[:, r],
                        xTp[b][:, off : off + P],
                        w1xT,
                        start=True,
                        stop=True,
                    )
                if qq % 2 == 0:
                    nc.vector.tensor_copy(xmsg[:, 4 * qq : 4 * qq + 4], xm)
                else:
                    nc.scalar.copy(xmsg[:, 4 * qq : 4 * qq + 4], xm)

            # ---------------- main loop ----------------
            agg = pp.tile([P, rows], F32, tag="agg", bufs=1, name="agg")
            # self-connection term opens each half's accumulation group
            # (a matmul may not span a PSUM bank -> 512-col halves)
            for h in range(2):
                sl = slice(h * 512, (h + 1) * 512)
                nc.tensor.matmul(
                    agg[:, sl], w2T, xTs_sb[:, sl], start=True, stop=False
                )

            def do_pair(q, et2, dd):
                """DVE/Scalar work for one pair; returns prod tile."""
                if is_cast(q):
                    db = dd  # already bf16 via casting dma
                else:
                    db = pool.tile([P, 2, 2, rows], BF16, tag="DB", bufs=3,
                                   name=f"db{q}")
                    nc.scalar.copy(db, dd)
                prod = pool.tile([P, 2, 2, rows], BF16, tag="PR",
                                 bufs=2 * DIST_BATCH, name=f"prod{q}")
                ebc = et2[:, :, None, :].broadcast_to((P, 2, 2, rows))
                nc.vector.tensor_tensor(prod, ebc, db, op=mybir.AluOpType.mult)
                return prod

            # xmsg batch g feeds pair-group g; keep 2 groups of lookahead
            xmsg_stage(0)
            xmsg_stage(1)

            for g in range(n_grp):
                group = range(g * DIST_BATCH, (g + 1) * DIST_BATCH)
                prods = []
                for q in group:
                    et2, dd = pre.pop(q) if q in pre else load_pair(q)
                    prods.append(do_pair(q, et2, dd))
                    # agg matmuls (stationary = xmsg chunk, reused for
                    # both 512-col halves)
                    for t in range(2):
                        for h in range(2):
                            sl = slice(h * 512, (h + 1) * 512)
                            nc.tensor.matmul(
                                agg[:, sl],
                                xmsg[:, 2 * q + t],
                                et2[:, t, sl],
                                start=False,
                                stop=False,
                            )
                if g + 2 < n_grp:
                    xmsg_stage(g + 2)
                for c in range(2):
                    for k, q in enumerate(group):
                        for t in range(2):
                            for h in range(2):
                                sl = slice(h * 512, (h + 1) * 512)
                                nc.tensor.matmul(
                                    agg[:, sl],
                                    w1db[:, c],
                                    prods[k][:, t, c, sl],
                                    start=False,
                                    stop=c == 1 and q == n_pair - 1 and t == 1,
                                )

            # ---------------- epilogue ----------------
            out_sb = pool.tile([P, rows], F32, tag="osb", bufs=1)
            for h in range(2):
                sl = slice(h * 512, (h + 1) * 512)
                nc.scalar.copy(out_sb[:, sl], agg[:, sl])
                nc.sync.dma_start(o_d[:, sl], out_sb[:, sl])

    nc.compile()
    return nc


def _prep_inputs(inputs, n, rows_per_core, n_cores):
    """Host-side shard + layout + dtype prep (numpy only)."""
    bf16 = ml_dtypes.bfloat16
    # e4m3fn: bit-identical to TRN FP8_EXP4 for |x| <= 240 (all our data),
    # and the fn variant is the one the PJRT plugin accepts as input type
    fp8 = ml_dtypes.float8_e4m3fn
    x = np.asarray(inputs["x"], dtype=np.float32)
    edges = np.asarray(inputs["edges"], dtype=np.float32)
    dist = np.asarray(inputs["distance_matrix"], dtype=np.float32)
    w1 = np.asarray(inputs["w1"], dtype=np.float32)
    w2 = np.asarray(inputs["w2"], dtype=np.float32)
    f = x.shape[1]
    R = rows_per_core

    xT = np.ascontiguousarray(x.T).astype(bf16)            # [f, n]
    w1xT = np.ascontiguousarray(w1[:, :f].T).astype(bf16)  # [k, F_OUT]
    w2T = np.ascontiguousarray(w2.T).astype(bf16)          # [k, F_OUT]
    # broadcast-w1d stationaries: w1db[j, c, fout] = w1[fout, f+c]
    w1db = np.ascontiguousarray(
        np.broadcast_to(w1[:, f : f + 2].T[None, :, :], (P, 2, w1.shape[0]))
    ).astype(bf16)

    in_maps = []
    for c in range(n_cores):
        i0 = c * R
        i1 = i0 + R
        # E^T slice [n, R], pair-packed to [n/256, 128, 2, R] so each
        # partition row carries 4 KB contiguous
        eT = edges[i0:i1].T.astype(bf16)
        eP = np.ascontiguousarray(
            eT.reshape(n // (2 * P), 2, P, R).transpose(0, 2, 1, 3)
        )
        # D^T channel-packed [n, 2, R] (dT[j, c, i] = dist[i, j, c]),
        # pair-packed to [n/256, 128, 2, 2, R]
        dT = dist[i0:i1].transpose(1, 2, 0).astype(fp8)
        dP = np.ascontiguousarray(
            dT.reshape(n // (2 * P), 2, P, 2, R).transpose(0, 2, 1, 3, 4)
        )
        in_maps.append(
            {
                "eP": eP,
                "dP": dP,
                "xT": xT,
                "xT_self": np.ascontiguousarray(xT[:, i0:i1]),
                "w1xT": w1xT,
                "w2T": w2T,
                "w1db": w1db,
            }
        )
    return in_maps


def _run(inputs, n, rows_per_core, n_cores, trace=False):
    from concourse.bass_utils import run_bass_kernel_spmd

    in_maps = _prep_inputs(inputs, n, rows_per_core, n_cores)
    nc = build(n=n, rows=rows_per_core)
    res = run_bass_kernel_spmd(nc, in_maps, core_ids=list(range(n_cores)), trace=trace)

    global LAST_RESULT
    LAST_RESULT = res

    out = np.concatenate([r["outT"].T for r in res.results], axis=0)
    return np.ascontiguousarray(out, dtype=np.float32)


def kernel(**inputs) -> np.ndarray:
    trace = os.environ.get("KERNEL_TRACE", "0") == "1"
    return _run(
        inputs,
        n=N_FULL,
        rows_per_core=N_FULL // N_CORES,
        n_cores=N_CORES,
        trace=trace,
    )
